# revision 19
# baseline (speedup 1.0000x reference)
"""Multi-head attention (B=2, S=2048, D=1024, H=16, dk=64) on 8 trn2 cores.

Sharding: data-parallel over batch (2) x tensor-parallel over heads (4 groups
of 4 heads).  Core c handles batch c//4, heads (c%4)*4 .. +4.  Each core
computes its 4 heads' Q/K/V projections, attention, and its slice of the
output projection (Wo row-parallel); the host sums the 4 partial outputs per
batch and adds bo.

Host-side prep (outside HW timing):
  - keys/values are packed by v_mask (masked keys dropped, padded to a
    multiple of 128); padding keys get an additive -30000 exp bias -> 0.
  - all inputs are cast to bf16 and laid out as their exact SBUF images
    [128, X] so every tensor loads with a few large row-efficient DMAs.

Device per core (matmuls bf16 -> fp32 PSUM):
  kwT/qwT[hp] [128, S*]: head-pair projections, d' on partitions; bq/bk
    folded in via DVE tensor_scalar_add on the PSUM->SBUF evacuation.
  vw assembled into AV-lhsT tiles avl[hp][jt] [128, 193] with embedded
    ones/zeros columns (denominator rides the AV matmul for free).
  attention per (ic 512-query chunk, hp): per key tile jt:
    s[:, :512] / s[:, 512:] via two concurrent K=64 row-tiled matmuls,
    ONE exp ACTIVATE [128, 1024] (scale=1/8, per-key mask bias),
    AV accumulate into u_lo/u_hi PSUM.
  normalization: denominator rows -> ones-matmul broadcast to 128
    partitions -> 1/D = exp(-ln D) on the ACT engine (rides the exp
    stream; ln+exp share one ACT table set) -> two tensor_muls into uTn.
  out[s, e] = sum_f uTn[f, s] Wo[f, e] interleaved with attention; the
  Qproj/Wo matmuls fill PE gaps.

Schedule: input DMAs are split across two issue queues with WAW gates
(tiny gpsimd copies into each dest tile) so the critical set
(wk/wq/qt0/kt) gets full HBM bandwidth before vt/wo/qt1-3 start; Kproj
runs chunk-outer kt-inner on the score-PSUM slots; Qproj(ic0,hp1) and
Vproj (kt-outer waves, one PSUM bank per jt) are closures woven into
block (0,0); each block's AV-tail + normalization is deferred into the
next block's score/exp stream (cross-block software pipelining); the
last ic's Wo hp0 pass rides the final block as fillers and only a short
hp1+add pass remains in the tail.
"""

import math

import numpy as np
import ml_dtypes

BF16 = np.dtype(ml_dtypes.bfloat16)

HEADS = 16
DK = 64
D = 1024
S = 2048
B = 2
NCORES = 8
HPC = 4          # heads per core
CH = HPC * DK    # 256 = d' slice per core
P = 128
NKT = D // P     # 8 contraction tiles
QC = 512         # query chunk (attention block width)
NIC = S // QC    # 4
NEG = -30000.0   # additive bias that drives exp() to exactly 0

_NC_CACHE = {}


def _split_multi_waits(nc, mybir):
    """This toolchain's walrus allows only ONE sync wait per instruction.
    Hoist extra waits into standalone EventSemaphore instructions."""
    for f in nc.m.functions:
        for bb in f.blocks:
            il = bb.instructions
            i = 0
            while i < len(il):
                inst = il[i]
                si = inst.sync_info
                waits = list(si.on_wait) if (si and si.on_wait) else []
                if len(waits) > 1:
                    for k, w in enumerate(waits[:-1]):
                        ev = mybir.InstEventSemaphore(
                            name=f"{inst.name}-hw{k}",
                            engine=inst.engine,
                            ins=[], outs=[],
                            sync_info=mybir.SyncInfo(on_wait=[w],
                                                     on_update=[]),
                        )
                        il.insert(i, ev)
                        i += 1
                    si.on_wait = [waits[-1]]
                    inst.sync_info = si
                i += 1


def build_nc(skp, legalize=True):
    """Build the single-core Bass program (SPMD across the 8 cores).

    Schedule (vs the v0 baseline): no long warmup block; Qproj(ic0) and
    Kproj start as their DMAs land (Kproj kt-outer, DMA-paced); Vproj runs
    kt-outer in two 2-bank waves emitted as closures woven into block
    (0,0); each attention block's AV-tail + normalization is deferred into
    the NEXT block's score/exp stream (cross-block software pipelining) so
    the ACT engine never starves at block boundaries.  Normalization uses
    reciprocal_approx_fast and all PSUM->SBUF copies run on DVE, keeping
    ACT exp-only.

    PSUM banks: "s" x2 (scores / Kproj accumulators, 2 banks each),
    "w1"/"w2" (warmup, Qproj ic0, Vproj waves, norm broadcast, Wo/Qproj
    fillers; 1 bank each), "ulo"/"uhi" (Kproj remainder, AV accumulators).
    """
    if skp // P > 9:
        return build_nc_v0(skp, legalize)
    from collections import deque

    import concourse.bass as bass
    import concourse.mybir as mybir
    import concourse.tile as tile

    f32 = mybir.dt.float32
    bf16 = mybir.dt.bfloat16
    njt = skp // P
    Exp = mybir.ActivationFunctionType.Exp
    Log = mybir.ActivationFunctionType.Ln if hasattr(
        mybir.ActivationFunctionType, "Ln") else (
        mybir.ActivationFunctionType.Log)

    nc = bass.Bass()
    kt_d = nc.declare_dram_parameter("kt", [P, NKT * skp], bf16, isOutput=False)
    vt_d = nc.declare_dram_parameter("vt", [P, NKT * skp], bf16, isOutput=False)
    qt_d = nc.declare_dram_parameter("qt", [P, NIC * NKT * QC], bf16,
                                     isOutput=False)
    wq_d = nc.declare_dram_parameter("wq", [P, NKT * CH], bf16, isOutput=False)
    wk_d = nc.declare_dram_parameter("wk", [P, NKT * CH], bf16, isOutput=False)
    wv_d = nc.declare_dram_parameter("wv", [P, NKT * CH], bf16, isOutput=False)
    wvb_d = nc.declare_dram_parameter("wvb", [1, CH], bf16, isOutput=False)
    wo_d = nc.declare_dram_parameter("wo", [P, 2 * D], bf16, isOutput=False)
    bqk_d = nc.declare_dram_parameter("bqk", [P, 4], f32, isOutput=False)
    mb_d = nc.declare_dram_parameter("mb", [P, njt], f32, isOutput=False)
    out_d = nc.declare_dram_parameter("out", [S, D], f32, isOutput=True)

    with tile.TileContext(nc) as tc:
        with (
            tc.tile_pool(name="consts", bufs=1) as consts,
            tc.tile_pool(name="proj", bufs=1) as proj,
            tc.tile_pool(name="ptile", bufs=9) as ptile,
            tc.tile_pool(name="norm", bufs=2) as norm,
            tc.tile_pool(name="outp", bufs=4) as outp,
            tc.tile_pool(name="psum", bufs=1, space="PSUM") as psum,
            tc.tile_pool(name="spsum", bufs=2, space="PSUM") as spsum,
        ):
            # ---- input DMAs, ordered by when the schedule needs them ----
            bqk_t = consts.tile([P, 4], f32, tag="bqk", name="bqk_t")
            nc.sync.dma_start(out=bqk_t[:, :], in_=bqk_d[:, :])
            mb_t = consts.tile([P, njt], f32, tag="mb", name="mb_t")
            nc.sync.dma_start(out=mb_t[:, :], in_=mb_d[:, :])
            wk_t = consts.tile([P, NKT * CH], bf16, tag="wk", name="wk_t")
            nc.sync.dma_start(out=wk_t[:, :], in_=wk_d[:, :])
            wq_t = consts.tile([P, NKT * CH], bf16, tag="wq", name="wq_t")
            nc.sync.dma_start(out=wq_t[:, :], in_=wq_d[:, :])
            qt_t = [consts.tile([P, NKT * QC], bf16, tag=f"qt{ic}",
                                name=f"qt{ic}") for ic in range(NIC)]
            nc.sync.dma_start(out=qt_t[0][:, :], in_=qt_d[:, 0:NKT * QC])
            kt_t = []
            for kt in range(NKT):
                t = consts.tile([P, skp], bf16, tag=f"kt{kt}", name=f"kt{kt}")
                nc.sync.dma_start(out=t[:, :],
                                  in_=kt_d[:, kt * skp:(kt + 1) * skp])
                kt_t.append(t)
            # Non-critical transfers ride the gpsimd queue, each gated by a
            # tiny copy into its dest tile from a tensor that must land
            # first: the dma_start inherits the ordering via WAW even if the
            # scheduler reorders the queue.  Sequencing keeps the critical
            # set (wk/kt/wq/qt0) at full HBM bandwidth.
            def gated_dma(dst_ap, src_ap, gate_ap):
                nc.gpsimd.tensor_copy(dst_ap[0:1, 0:1], gate_ap[0:1, 0:1])
                nc.gpsimd.dma_start(out=dst_ap, in_=src_ap)

            g0 = kt_t[NKT - 1]  # vt starts after the critical set
            wv_t = consts.tile([P, NKT * CH], bf16, tag="wv", name="wv_t")
            gated_dma(wv_t[:, :], wv_d[:, :], g0)
            wvb_t = consts.tile([1, CH], bf16, tag="wvb", name="wvb_t")
            nc.gpsimd.dma_start(out=wvb_t[:, :], in_=wvb_d[:, :])
            vt_t = []
            for kt in range(NKT):
                t = consts.tile([P, skp], bf16, tag=f"vt{kt}", name=f"vt{kt}")
                gated_dma(t[:, :], vt_d[:, kt * skp:(kt + 1) * skp], g0)
                vt_t.append(t)
            g1 = vt_t[NKT - 1]
            qt_ds = [qt_d[:, ic * NKT * QC:(ic + 1) * NKT * QC]
                     for ic in range(NIC)]
            gated_dma(qt_t[1][:, :], qt_ds[1], g1)
            wo_t = consts.tile([P, 2 * D], bf16, tag="wo", name="wo_t")
            gated_dma(wo_t[:, :], wo_d[:, :], g1)
            gated_dma(qt_t[2][:, :], qt_ds[2], qt_t[1])
            gated_dma(qt_t[3][:, :], qt_ds[3], qt_t[2])

            ones_t = consts.tile([1, P], bf16, tag="ones", name="ones_t")
            nc.vector.memset(ones_t[:, :], 1.0)
            wu = consts.tile([P, 512], bf16, tag="wu", name="wu")
            nc.vector.memset(wu[:, :], 0.0)

            # AV lhsT tiles: per (hp, jt) [128, 193]:
            #   lo lhsT = avl[:, 0:65]   = [vw_lo | ones]
            #   hi lhsT = avl[:, 65:193] = [ones | zeros(63) | vw_hi]
            avl = [[None] * njt for _ in range(2)]
            for hp in range(2):
                for jt in range(njt):
                    t = proj.tile([P, 193], bf16, tag=f"avl{hp}_{jt}",
                                  name=f"avl{hp}_{jt}")
                    nc.vector.memset(t[:, 64:129], 0.0)
                    nc.vector.memset(t[:, 64:66], 1.0)
                    avl[hp][jt] = t

            kwT = [proj.tile([P, skp], bf16, tag=f"kwT{hp}", name=f"kwT{hp}")
                   for hp in range(2)]
            qwT = [proj.tile([P, S], bf16, tag=f"qwT{hp}", name=f"qwT{hp}")
                   for hp in range(2)]
            uTn = [proj.tile([P, S], bf16, tag=f"uTn{hp}", name=f"uTn{hp}")
                   for hp in range(2)]

            # hoist the ACT exp-table load off the critical path
            actw = norm.tile([1, 4], f32, tag="actw", name="actw")
            nc.scalar.activation(actw[:, :], bqk_t[0:1, 0:4], Exp)

            # short PE warmup so HAM is hot when the projections start
            wps = psum.tile([P, 512], f32, tag="w1", name="warm")
            for _ in range(12):
                nc.tensor.matmul(wps[:, :], wu[:, 0:P], wu[:, :],
                                 start=True, stop=True)

            # ---- Qproj helpers ----
            def qproj_mms(ps, ic, hp, kts):
                for kt in kts:
                    nc.tensor.matmul(
                        ps[:, :],
                        wq_t[:, kt * CH + hp * P:kt * CH + (hp + 1) * P],
                        qt_t[ic][:, kt * QC:(kt + 1) * QC],
                        start=(kt == 0), stop=(kt == NKT - 1))

            def qproj_evac(ps, ic, hp):
                nc.vector.tensor_scalar_add(
                    qwT[hp][:, ic * QC:(ic + 1) * QC], ps[:, :],
                    bqk_t[:, hp:hp + 1])

            # Qproj (ic0, hp0): first thing after its DMAs land
            q0ps = psum.tile([P, QC], f32, tag="w2", name="q0ps")
            qproj_mms(q0ps, 0, 0, range(NKT))
            qproj_evac(q0ps, 0, 0)

            # ---- Kproj: chunk (0,hp0) runs inline on an "s" slot and
            # gates the first scores; the remaining (chunk,hp) groups are
            # closures woven into block (0,0), each on its own lazily
            # allocated ulo/uhi bank so "s"-slot recycling stays safe ----
            kch = []
            o = 0
            while o < skp:
                kch.append((o, min(512, skp - o)))
                o += 512

            def kproj_mms(dst, hp, co, cw):
                for kt in range(NKT):
                    nc.tensor.matmul(
                        dst,
                        wk_t[:, kt * CH + hp * P:kt * CH + (hp + 1) * P],
                        kt_t[kt][:, co:co + cw],
                        start=(kt == 0), stop=(kt == NKT - 1))
                nc.vector.tensor_scalar_add(kwT[hp][:, co:co + cw], dst,
                                            bqk_t[:, 2 + hp:3 + hp])

            kw = min(skp, 1024)
            kps = [spsum.tile([P, kw], f32, tag="s", name=f"kps{hp}")
                   for hp in range(2)]
            kpr = None
            if skp > 1024:
                kpr = [psum.tile([P, skp - 1024], f32, tag=t, name=f"kpr{h}")
                       for h, t in enumerate(("ulo", "uhi"))]

            def ksl(hp, co, cw):
                if co < 1024:
                    return kps[hp][:, co:co + cw]
                return kpr[hp][:, 0:cw]

            for hp in (0, 1):
                for co, cw in kch:
                    kproj_mms(ksl(hp, co, cw), hp, co, cw)

            # ---- Vproj: kt-outer waves of 4 jts, one PSUM bank per jt
            # (interleaved accumulation groups may not share a bank), emitted
            # as closures woven into block (0,0) ----
            WTAGS = ("ulo", "uhi", "w1", "w2")
            vwave = {}

            def valloc(w):
                vwave[w] = [psum.tile([P, CH], f32, tag=WTAGS[j],
                                      name=f"vps{w}_{j}")
                            for j in range(min(4, njt - 4 * w))]

            def vsl(w, jt):
                return vwave[w][jt - 4 * w][:, :]

            def v_mms(w, kts, alloc_first=False):
                jts = range(4 * w, min(njt, 4 * w + 4))

                def f():
                    if alloc_first:
                        valloc(w)
                    for kt in kts:
                        for jt in jts:
                            nc.tensor.matmul(
                                vsl(w, jt),
                                vt_t[kt][:, jt * P:(jt + 1) * P],
                                wv_t[:, kt * CH:(kt + 1) * CH],
                                start=(kt == 0), stop=False)
                return f

            def v_bias(w):
                def f():
                    for jt in range(4 * w, min(njt, 4 * w + 4)):
                        nc.tensor.matmul(vsl(w, jt), ones_t[0:1, :],
                                         wvb_t[0:1, :], start=False,
                                         stop=True)
                return f

            def v_evac(w, hp):
                def f():
                    for jt in range(4 * w, min(njt, 4 * w + 4)):
                        nc.vector.tensor_copy(
                            avl[hp][jt][:, 0:64],
                            vsl(w, jt)[:, hp * P:hp * P + 64])
                        nc.vector.tensor_copy(
                            avl[hp][jt][:, 129:193],
                            vsl(w, jt)[:, hp * P + 64:(hp + 1) * P])
                return f

            def fuse(*fns):
                def f():
                    for g in fns:
                        g()
                return f

            pre_tail = deque()
            qbox = {}

            def qp1a():
                qbox["ps"] = psum.tile([P, QC], f32, tag="w1", name="q1ps")
                qproj_mms(qbox["ps"], 0, 1, range(4))

            def qp1b():
                qproj_mms(qbox["ps"], 0, 1, range(4, NKT))
                qproj_evac(qbox["ps"], 0, 1)

            pre_tail.append(qp1a)
            pre_tail.append(qp1b)
            # wave 0 (jts 0-3): DMA-paced, bias+first-evac fused into the
            # last kt closure so avl[hp0][0..3] exists before AV0
            pre_tail.append(v_mms(0, [0], alloc_first=True))
            for kt in range(1, NKT - 1):
                pre_tail.append(v_mms(0, [kt]))
            pre_tail.append(fuse(v_mms(0, [NKT - 1]), v_bias(0), v_evac(0, 0)))
            pre_tail.append(v_evac(0, 1))
            # wave 1 (jts 4-7) + njt=9 trailer (jt 8): SBUF-fed, emitted
            # as per-kt granules so they never block a pending score pair
            pre_tail.append(v_mms(1, [0], alloc_first=True))
            for kt in range(1, NKT):
                pre_tail.append(v_mms(1, [kt]))
            pre_tail.append(fuse(v_bias(1), v_evac(1, 0)))
            pre_tail.append(v_evac(1, 1))
            if njt > 8:
                pre_tail.append(v_mms(2, range(NKT), alloc_first=True))
                pre_tail.append(fuse(v_bias(2), v_evac(2, 0), v_evac(2, 1)))

            # ---- fillers: Qproj(ic>=1) and Wo woven into attention ----
            fillers = deque()

            def qproj_units(ic):
                units = []
                for hp in range(2):
                    box = {}

                    def u(ic=ic, hp=hp, box=box, k0=0):
                        if k0 == 0:
                            box["ps"] = psum.tile([P, QC], f32, tag="w2",
                                                  name=f"qps{ic}_{hp}")
                        qproj_mms(box["ps"], ic, hp, range(k0, k0 + 2))
                        if k0 + 2 == NKT:
                            qproj_evac(box["ps"], ic, hp)

                    for k0 in range(0, NKT, 2):
                        units.append(lambda u=u, k0=k0: u(k0=k0))
                return units

            def wo_split_units(ic, hp, obs, tags=("w1", "w2")):
                units = []
                for st in range(QC // P):
                    s0 = ic * QC + st * P
                    sc = slice(s0, s0 + P)

                    def u(sc=sc, st=st, e=0):
                        if hp == 0 and e == 0:
                            obs[st] = outp.tile([P, D], f32, tag="ob",
                                                name="ob")
                        ob = obs[st]
                        ps = psum.tile([P, 512], f32,
                                       tag=tags[(st * 2 + e) % len(tags)],
                                       name="wops")
                        nc.tensor.matmul(ps[:, :], uTn[hp][:, sc],
                                         wo_t[:, hp * D + e * 512:
                                              hp * D + (e + 1) * 512],
                                         start=True, stop=True)
                        osl = ob[:, e * 512:(e + 1) * 512]
                        if hp == 0:
                            nc.vector.tensor_copy(osl, ps[:, :])
                        else:
                            nc.vector.tensor_add(osl, ps[:, :], osl)
                            nc.sync.dma_start(
                                out=out_d[sc, e * 512:(e + 1) * 512],
                                in_=osl)

                    units.append(lambda u=u: u(e=0))
                    units.append(lambda u=u: u(e=1))
                return units

            def wo_units(ic, tags=("w1",)):
                units = []
                for st in range(QC // P):
                    s0 = ic * QC + st * P
                    sc = slice(s0, s0 + P)
                    box = {}

                    def u(sc=sc, box=box, st=st, tags=tags, e=0, last=False):
                        if e == 0:
                            box["ob"] = outp.tile([P, D], f32, tag="ob",
                                                  name="ob")
                        ps = psum.tile([P, 512], f32,
                                       tag=tags[(st * 2 + e) % len(tags)],
                                       name="wops")
                        nc.tensor.matmul(ps[:, :], uTn[0][:, sc],
                                         wo_t[:, e * 512:(e + 1) * 512],
                                         start=True, stop=False)
                        nc.tensor.matmul(
                            ps[:, :], uTn[1][:, sc],
                            wo_t[:, D + e * 512:D + (e + 1) * 512],
                            start=False, stop=True)
                        nc.vector.tensor_copy(
                            box["ob"][:, e * 512:(e + 1) * 512], ps[:, :])
                        if last:
                            nc.sync.dma_start(out=out_d[sc, :],
                                              in_=box["ob"][:, :])

                    units.append(lambda u=u: u(e=0, last=False))
                    units.append(lambda u=u: u(e=1, last=True))
                return units

            # ---- attention block with cross-block tail pipelining ----
            def attn_block(ic, hp, prev_tail, lag, tail_fast=True):
                icq = slice(ic * QC, (ic + 1) * QC)
                ubox = {}
                pend = []

                def av(jt, pt):
                    first, last = (jt == 0), (jt == njt - 1)
                    if first:
                        ubox["lo"] = psum.tile([P, QC], f32, tag="ulo",
                                               name="u_lo")
                        ubox["hi"] = psum.tile([P, QC], f32, tag="uhi",
                                               name="u_hi")
                    u_lo, u_hi = ubox["lo"], ubox["hi"]
                    nc.tensor.matmul(u_lo[0:65, :], avl[hp][jt][:, 0:65],
                                     pt[:, 0:QC], start=first, stop=last)
                    nc.tensor.matmul(u_hi[:, :], avl[hp][jt][:, 65:193],
                                     pt[:, QC:2 * QC], start=first, stop=last)

                for jt in range(njt):
                    jc = slice(jt * P, (jt + 1) * P)
                    s = spsum.tile([P, 2 * QC], f32, tag="s", name="s")
                    nc.tensor.matmul(s[:, 0:QC], kwT[hp][0:64, jc],
                                     qwT[hp][0:64, icq],
                                     start=True, stop=True)
                    nc.tensor.matmul(s[:, QC:2 * QC], kwT[hp][64:128, jc],
                                     qwT[hp][64:128, icq],
                                     start=True, stop=True)
                    pt = ptile.tile([P, 2 * QC], bf16, tag="p", name="pt")
                    nc.scalar.activation(pt[:, :], s[:, :], Exp,
                                         bias=mb_t[:, jt:jt + 1], scale=0.125)
                    pend.append((jt, pt))
                    if prev_tail:
                        npop = 3 if (tail_fast and jt < 3) else 2
                        for _ in range(npop):
                            if prev_tail:
                                prev_tail.popleft()()
                    elif jt >= 3 and fillers:
                        fillers.popleft()()
                    if len(pend) > lag:
                        while prev_tail:  # frees the u/w banks for our AVs
                            prev_tail.popleft()()
                        av(*pend.pop(0))
                        if not prev_tail and fillers and jt >= 6:
                            fillers.popleft()()
                # leftover non-hazard prologue work drains here
                while prev_tail:
                    prev_tail.popleft()()

                tail = deque()
                for jt, pt in pend:
                    tail.append(lambda jt=jt, pt=pt: av(jt, pt))
                drl = norm.tile([1, QC], bf16, tag="drl", name="drl")
                drh = norm.tile([1, QC], bf16, tag="drh", name="drh")
                nbox = {}

                def n1():
                    nc.vector.tensor_copy(drl[:, :], ubox["lo"][64:65, :])
                    nc.vector.tensor_copy(drh[:, :], ubox["hi"][0:1, :])

                def n2():
                    bc = psum.tile([P, QC], f32, tag="w1", name="bc")
                    nbox["bc"] = bc
                    nc.tensor.matmul(bc[0:64, :], ones_t[0:1, 0:64],
                                     drl[0:1, :], start=True, stop=True)
                    nc.tensor.matmul(bc[64:128, :], ones_t[0:1, 0:64],
                                     drh[0:1, :], start=True, stop=True,
                                     skip_group_check=True)

                def n3a():
                    # 1/D = exp(-ln D): both live in the natural_log_exp
                    # ACT table set, so this rides the exp stream with no
                    # DVE reciprocal and near-zero added latency
                    lnb = norm.tile([P, QC], f32, tag="lnb", name="lnb")
                    rbc = norm.tile([P, QC], f32, tag="rbc", name="rbc")
                    nbox["rbc"] = rbc
                    nc.scalar.activation(lnb[:, :], nbox["bc"][:, :], Log)
                    nc.scalar.activation(rbc[:, :], lnb[:, :], Exp,
                                         scale=-1.0)

                def n3b():
                    rbc = nbox["rbc"]
                    nc.vector.tensor_mul(uTn[hp][0:64, icq],
                                         ubox["lo"][0:64, :], rbc[0:64, :])
                    nc.vector.tensor_mul(uTn[hp][64:128, icq],
                                         ubox["hi"][64:128, :],
                                         rbc[64:128, :])

                tail.extend([n1, n2, n3a, n3b])
                return tail

            blocks = [(ic, hp) for ic in range(NIC) for hp in range(2)]
            prev = pre_tail
            wo3 = {}
            for bi, (ic, hp) in enumerate(blocks):
                if hp == 0 and ic >= 1:
                    fillers.extend(wo_units(ic - 1))
                if hp == 1 and ic + 1 < NIC:
                    fillers.extend(qproj_units(ic + 1))
                if bi == len(blocks) - 1:
                    # last ic's Wo hp0 pass rides the final block as fillers
                    fillers.extend(wo_split_units(NIC - 1, 0, wo3))
                prev = attn_block(ic, hp, prev,
                                  lag=(njt - 1) if bi == 0 else
                                  (2 if bi == len(blocks) - 1 else 3),
                                  tail_fast=(bi > 0))
            while prev:
                prev.popleft()()
            while fillers:
                fillers.popleft()()
            for u in wo_split_units(NIC - 1, 1, wo3):
                u()

    if legalize:
        _split_multi_waits(nc, mybir)
    return nc


def build_nc_v0(skp, legalize=True):
    """v0 baseline build (fallback for unexpected skp)."""
    import concourse.bass as bass
    import concourse.mybir as mybir
    import concourse.tile as tile

    f32 = mybir.dt.float32
    bf16 = mybir.dt.bfloat16
    njt = skp // P
    Exp = mybir.ActivationFunctionType.Exp
    Log = mybir.ActivationFunctionType.Ln if hasattr(
        mybir.ActivationFunctionType, "Ln") else (
        mybir.ActivationFunctionType.Log)

    nc = bass.Bass()
    kt_d = nc.declare_dram_parameter("kt", [P, NKT * skp], bf16, isOutput=False)
    vt_d = nc.declare_dram_parameter("vt", [P, NKT * skp], bf16, isOutput=False)
    qt_d = nc.declare_dram_parameter("qt", [P, NIC * NKT * QC], bf16,
                                     isOutput=False)
    wq_d = nc.declare_dram_parameter("wq", [P, NKT * CH], bf16, isOutput=False)
    wk_d = nc.declare_dram_parameter("wk", [P, NKT * CH], bf16, isOutput=False)
    wv_d = nc.declare_dram_parameter("wv", [P, NKT * CH], bf16, isOutput=False)
    wvb_d = nc.declare_dram_parameter("wvb", [1, CH], bf16, isOutput=False)
    wo_d = nc.declare_dram_parameter("wo", [P, 2 * D], bf16, isOutput=False)
    bqk_d = nc.declare_dram_parameter("bqk", [P, 4], f32, isOutput=False)
    mb_d = nc.declare_dram_parameter("mb", [P, njt], f32, isOutput=False)
    out_d = nc.declare_dram_parameter("out", [S, D], f32, isOutput=True)

    def chunks(total, width):
        c = []
        o = 0
        while o < total:
            c.append((o, min(width, total - o)))
            o += width
        return c

    with tile.TileContext(nc) as tc:
        with (
            tc.tile_pool(name="consts", bufs=1) as consts,
            tc.tile_pool(name="proj", bufs=1) as proj,
            tc.tile_pool(name="ptile", bufs=6) as ptile,
            tc.tile_pool(name="norm", bufs=2) as norm,
            tc.tile_pool(name="outp", bufs=4) as outp,
            tc.tile_pool(name="psum", bufs=1, space="PSUM") as psum,
            tc.tile_pool(name="spsum", bufs=2, space="PSUM") as spsum,
        ):
            # ---- input DMAs (ordered by first use) ----
            kt_t = []
            for kt in range(NKT):
                t = consts.tile([P, skp], bf16, tag=f"kt{kt}", name=f"kt{kt}")
                nc.sync.dma_start(out=t[:, :],
                                  in_=kt_d[:, kt * skp:(kt + 1) * skp])
                kt_t.append(t)
            wk_t = consts.tile([P, NKT * CH], bf16, tag="wk", name="wk_t")
            nc.sync.dma_start(out=wk_t[:, :], in_=wk_d[:, :])
            wq_t = consts.tile([P, NKT * CH], bf16, tag="wq", name="wq_t")
            nc.sync.dma_start(out=wq_t[:, :], in_=wq_d[:, :])
            bqk_t = consts.tile([P, 4], f32, tag="bqk", name="bqk_t")
            nc.sync.dma_start(out=bqk_t[:, :], in_=bqk_d[:, :])
            mb_t = consts.tile([P, njt], f32, tag="mb", name="mb_t")
            nc.sync.dma_start(out=mb_t[:, :], in_=mb_d[:, :])
            qt_t = []
            for ic in range(NIC):
                t = consts.tile([P, NKT * QC], bf16, tag=f"qt{ic}",
                                name=f"qt{ic}")
                qt_t.append(t)
            nc.sync.dma_start(out=qt_t[0][:, :], in_=qt_d[:, 0:NKT * QC])
            wv_t = consts.tile([P, NKT * CH], bf16, tag="wv", name="wv_t")
            nc.gpsimd.dma_start(out=wv_t[:, :], in_=wv_d[:, :])
            wvb_t = consts.tile([1, CH], bf16, tag="wvb", name="wvb_t")
            nc.gpsimd.dma_start(out=wvb_t[:, :], in_=wvb_d[:, :])
            vt_t = []
            for kt in range(NKT):
                t = consts.tile([P, skp], bf16, tag=f"vt{kt}", name=f"vt{kt}")
                nc.gpsimd.dma_start(out=t[:, :],
                                    in_=vt_d[:, kt * skp:(kt + 1) * skp])
                vt_t.append(t)
            wo_t = consts.tile([P, 2 * D], bf16, tag="wo", name="wo_t")
            nc.gpsimd.dma_start(out=wo_t[:, :], in_=wo_d[:, :])
            for ic in range(1, NIC):
                nc.gpsimd.dma_start(
                    out=qt_t[ic][:, :],
                    in_=qt_d[:, ic * NKT * QC:(ic + 1) * NKT * QC])

            ones_t = consts.tile([1, P], bf16, tag="ones", name="ones_t")
            nc.vector.memset(ones_t[:, :], 1.0)

            # PE warmup: dense junk matmuls spanning the DMA preload so HAM
            # is at K=8/8 when the real projections start
            wu = consts.tile([P, 512], bf16, tag="wu", name="wu")
            nc.vector.memset(wu[:, :], 0.0)
            wps = spsum.tile([P, QC], f32, tag="s", name="warm")
            for _ in range(56):
                nc.tensor.matmul(wps[:, :], wu[:, 0:P], wu[:, :],
                                 start=True, stop=True)

            # AV lhsT tiles: per (hp, jt) [128, 193]:
            #   lo lhsT = avl[:, 0:65]   = [vw_lo | ones]
            #   hi lhsT = avl[:, 65:193] = [ones | zeros(63) | vw_hi]
            avl = [[None] * njt for _ in range(2)]
            for hp in range(2):
                for jt in range(njt):
                    t = proj.tile([P, 193], bf16, tag=f"avl{hp}_{jt}",
                                  name=f"avl{hp}_{jt}")
                    nc.vector.memset(t[:, 64:129], 0.0)
                    nc.vector.memset(t[:, 64:66], 1.0)
                    avl[hp][jt] = t

            kwT = [proj.tile([P, skp], bf16, tag=f"kwT{hp}", name=f"kwT{hp}")
                   for hp in range(2)]
            qwT = [proj.tile([P, S], bf16, tag=f"qwT{hp}", name=f"qwT{hp}")
                   for hp in range(2)]
            uTn = [proj.tile([P, S], bf16, tag=f"uTn{hp}", name=f"uTn{hp}")
                   for hp in range(2)]

            # ---- K projection (chunk-outer, kt-inner) ----
            for co, cw in chunks(skp, 512):
                for hp in range(2):
                    ps = psum.tile([P, 512], f32, tag=("ulo", "uhi")[hp],
                                   name="kps")
                    for kt in range(NKT):
                        nc.tensor.matmul(
                            ps[:, :cw],
                            wk_t[:, kt * CH + hp * P:kt * CH + (hp + 1) * P],
                            kt_t[kt][:, co:co + cw],
                            start=(kt == 0), stop=(kt == NKT - 1))
                    nc.vector.tensor_scalar_add(kwT[hp][:, co:co + cw],
                                                ps[:, :cw],
                                                bqk_t[:, 2 + hp:3 + hp])

            # ---- V projection (kt-outer in passes of 2 key tiles) ----
            for j0 in range(0, njt, 2):
                jts = list(range(j0, min(j0 + 2, njt)))
                vps = {jt: psum.tile([P, CH], f32, tag=("bc", "mm")[jt - j0],
                                     name=f"vp{jt}") for jt in jts}
                for kt in range(NKT):
                    for jt in jts:
                        nc.tensor.matmul(
                            vps[jt][:, :],
                            vt_t[kt][:, jt * P:(jt + 1) * P],
                            wv_t[:, kt * CH:(kt + 1) * CH],
                            start=(kt == 0), stop=False)
                for jt in jts:
                    nc.tensor.matmul(vps[jt][:, :], ones_t[0:1, :],
                                     wvb_t[0:1, :], start=False, stop=True)
                for jt in jts:
                    for hp in range(2):
                        nc.scalar.copy(avl[hp][jt][:, 0:64],
                                       vps[jt][:, hp * P:hp * P + 64])
                        nc.scalar.copy(avl[hp][jt][:, 129:193],
                                       vps[jt][:, hp * P + 64:(hp + 1) * P])

            # ---- Q projection: chunk ic, head pair hp, one kt range ----
            def qproj_mms(ps, ic, hp, kts):
                for kt in kts:
                    nc.tensor.matmul(
                        ps[:, :],
                        wq_t[:, kt * CH + hp * P:kt * CH + (hp + 1) * P],
                        qt_t[ic][:, kt * QC:(kt + 1) * QC],
                        start=(kt == 0), stop=(kt == NKT - 1))

            def qproj_evac(ps, ic, hp):
                nc.vector.tensor_scalar_add(
                    qwT[hp][:, ic * QC:(ic + 1) * QC], ps[:, :],
                    bqk_t[:, hp:hp + 1])

            # qproj(0) runs pre-attention on the (still free) "s" slots
            for hp in range(2):
                ps = spsum.tile([P, QC], f32, tag="s", name=f"qps0_{hp}")
                qproj_mms(ps, 0, hp, range(NKT))
                qproj_evac(ps, 0, hp)

            def qproj_units(ic):
                units = []
                for hp in range(2):
                    box = {}

                    def u1(ic=ic, hp=hp, box=box):
                        box["ps"] = psum.tile([P, QC], f32, tag="mm",
                                              name=f"qps{ic}_{hp}")
                        qproj_mms(box["ps"], ic, hp, range(4))

                    def u2(ic=ic, hp=hp, box=box):
                        qproj_mms(box["ps"], ic, hp, range(4, NKT))
                        qproj_evac(box["ps"], ic, hp)

                    units += [u1, u2]
                return units

            def wo_units(ic, tags=("bc",)):
                units = []
                for st in range(QC // P):
                    s0 = ic * QC + st * P
                    sc = slice(s0, s0 + P)
                    box = {}

                    def u(sc=sc, box=box, st=st, tags=tags, e=0, last=False):
                        if e == 0:
                            box["ob"] = outp.tile([P, D], f32, tag="ob",
                                                  name="ob")
                        ps = psum.tile([P, 512], f32,
                                       tag=tags[(st * 2 + e) % len(tags)],
                                       name="wops")
                        nc.tensor.matmul(ps[:, :], uTn[0][:, sc],
                                         wo_t[:, e * 512:(e + 1) * 512],
                                         start=True, stop=False)
                        nc.tensor.matmul(
                            ps[:, :], uTn[1][:, sc],
                            wo_t[:, D + e * 512:D + (e + 1) * 512],
                            start=False, stop=True)
                        nc.vector.tensor_copy(
                            box["ob"][:, e * 512:(e + 1) * 512], ps[:, :])
                        if last:
                            nc.sync.dma_start(out=out_d[sc, :],
                                              in_=box["ob"][:, :])

                    units.append(lambda u=u: u(e=0, last=False))
                    units.append(lambda u=u: u(e=1, last=True))
                return units

            # ---- attention with fillers woven between key tiles ----
            def attn_block(ic, hp, fillers):
                icq = slice(ic * QC, (ic + 1) * QC)
                u_lo = psum.tile([P, QC], f32, tag="ulo", name="u_lo")
                u_hi = psum.tile([P, QC], f32, tag="uhi", name="u_hi")

                def av(jt, pt):
                    first, last = (jt == 0), (jt == njt - 1)
                    nc.tensor.matmul(u_lo[0:65, :], avl[hp][jt][:, 0:65],
                                     pt[:, 0:QC], start=first, stop=last)
                    nc.tensor.matmul(u_hi[:, :], avl[hp][jt][:, 65:193],
                                     pt[:, QC:2 * QC], start=first, stop=last)
                    if jt >= 2 and fillers:
                        fillers.popleft()()

                LAG = 3
                pend = []
                for jt in range(njt):
                    jc = slice(jt * P, (jt + 1) * P)
                    s = spsum.tile([P, 2 * QC], f32, tag="s", name="s")
                    nc.tensor.matmul(s[:, 0:QC], kwT[hp][0:64, jc],
                                     qwT[hp][0:64, icq],
                                     start=True, stop=True)
                    nc.tensor.matmul(s[:, QC:2 * QC], kwT[hp][64:128, jc],
                                     qwT[hp][64:128, icq],
                                     start=True, stop=True)
                    pt = ptile.tile([P, 2 * QC], bf16, tag="p", name="pt")
                    nc.scalar.activation(pt[:, :], s[:, :], Exp,
                                         bias=mb_t[:, jt:jt + 1], scale=0.125)
                    pend.append((jt, pt))
                    if len(pend) > LAG:
                        av(*pend.pop(0))
                for jp in pend:
                    av(*jp)
                # normalization: D_lo at u_lo[64], D_hi at u_hi[0]
                drl = norm.tile([1, QC], bf16, tag="drl", name="drl")
                drh = norm.tile([1, QC], bf16, tag="drh", name="drh")
                nc.scalar.copy(drl[:, :], u_lo[64:65, :])
                nc.scalar.copy(drh[:, :], u_hi[0:1, :])
                bc = psum.tile([P, QC], f32, tag="bc", name="bc")
                nc.tensor.matmul(bc[0:64, :], ones_t[0:1, 0:64], drl[0:1, :],
                                 start=True, stop=True)
                nc.tensor.matmul(bc[64:128, :], ones_t[0:1, 0:64],
                                 drh[0:1, :], start=True, stop=True,
                                 skip_group_check=True)
                rbc = norm.tile([P, QC], f32, tag="rbc", name="rbc")
                nc.vector.reciprocal(rbc[:, :], bc[:, :])
                nc.vector.tensor_mul(uTn[hp][0:64, icq], u_lo[0:64, :],
                                     rbc[0:64, :])
                nc.vector.tensor_mul(uTn[hp][64:128, icq], u_hi[64:128, :],
                                     rbc[64:128, :])

            from collections import deque
            fillers = deque()
            for ic in range(NIC):
                if ic + 1 < NIC:
                    fillers.extend(qproj_units(ic + 1))
                if ic >= 1:
                    fillers.extend(wo_units(ic - 1))
                attn_block(ic, 0, fillers)
                attn_block(ic, 1, fillers)
            fillers.extend(wo_units(NIC - 1, tags=("bc", "mm")))
            while fillers:
                fillers.popleft()()

    if legalize:
        _split_multi_waits(nc, mybir)
    return nc


def prep_inputs(q, k, v, v_mask, Wq, bq, Wk, bk, Wv, bv, Wo, bo):
    """Pack/transpose/cast on the host. Returns (skp, in_maps)."""
    q = np.asarray(q, np.float32)
    k = np.asarray(k, np.float32)
    v = np.asarray(v, np.float32)
    v_mask = np.asarray(v_mask)

    idxs = [np.nonzero(v_mask[b])[0] for b in range(B)]
    skp = max(P, int(math.ceil(max(len(ix) for ix in idxs) / P)) * P)
    njt = skp // P

    def sbuf_image(a):
        # [D, X] -> [128, NKT * X] with kt-major free layout
        X = a.shape[1]
        return np.ascontiguousarray(
            a.reshape(NKT, P, X).transpose(1, 0, 2).reshape(P, NKT * X)
        ).astype(BF16)

    per_batch = []
    for b in range(B):
        ix = idxs[b]
        cnt = len(ix)
        kp = np.zeros((skp, D), np.float32)
        vp = np.zeros((skp, D), np.float32)
        kp[:cnt] = k[b][ix]
        vp[:cnt] = v[b][ix]
        kt_all = sbuf_image(kp.T)
        vt_all = sbuf_image(vp.T)
        # qt: [D, S] -> per-ic-chunk kt-major [128, NIC * NKT * QC]
        qt = np.ascontiguousarray(
            q[b].T.reshape(NKT, P, NIC, QC).transpose(1, 2, 0, 3)
            .reshape(P, NIC * NKT * QC)).astype(BF16)
        mbias = np.full(skp, NEG, np.float32)
        mbias[:cnt] = 0.0
        mb = np.ascontiguousarray(mbias.reshape(njt, P).T)  # [128, njt]
        per_batch.append((kt_all, vt_all, qt, mb))

    Wq = np.asarray(Wq, np.float32)
    Wk = np.asarray(Wk, np.float32)
    Wv = np.asarray(Wv, np.float32)
    Wo = np.asarray(Wo, np.float32)
    bq = np.asarray(bq, np.float32)
    bk = np.asarray(bk, np.float32)
    bv = np.asarray(bv, np.float32)

    in_maps = []
    for c in range(NCORES):
        b = c // 4
        c0 = (c % 4) * CH
        kt_all, vt_all, qt, mb = per_batch[b]
        bqk = np.stack([bq[c0:c0 + P], bq[c0 + P:c0 + CH],
                        bk[c0:c0 + P], bk[c0 + P:c0 + CH]], axis=1)
        wo_all = np.ascontiguousarray(
            Wo[c0:c0 + CH, :].reshape(2, P, D).transpose(1, 0, 2)
            .reshape(P, 2 * D)).astype(BF16)
        in_maps.append({
            "kt": kt_all, "vt": vt_all, "qt": qt,
            "wq": sbuf_image(Wq[:, c0:c0 + CH]),
            "wk": sbuf_image(Wk[:, c0:c0 + CH]),
            "wv": sbuf_image(Wv[:, c0:c0 + CH]),
            "wvb": np.ascontiguousarray(bv[c0:c0 + CH]).reshape(1, CH)
                     .astype(BF16),
            "wo": wo_all,
            "bqk": np.ascontiguousarray(bqk, np.float32),
            "mb": mb,
        })
    return skp, in_maps


def combine_outputs(results, bo):
    out = np.zeros((B, S, D), np.float32)
    for c in range(NCORES):
        out[c // 4] += results[c]["out"]
    out += np.asarray(bo, np.float32)
    return out


def kernel(q, k, v, v_mask, Wq, bq, Wk, bk, Wv, bv, Wo, bo, _trace=False):
    from concourse.bass_utils import run_bass_kernel_spmd

    skp, in_maps = prep_inputs(q, k, v, v_mask, Wq, bq, Wk, bk, Wv, bv, Wo, bo)
    if skp not in _NC_CACHE:
        _NC_CACHE[skp] = build_nc(skp)
    nc = _NC_CACHE[skp]
    res = run_bass_kernel_spmd(nc, in_maps, list(range(NCORES)), trace=_trace)
    out = combine_outputs(res.results, bo)
    if _trace:
        kernel.last_result = res
    return out

